# revision 9
# baseline (speedup 1.0000x reference)
"""PointConvolution (8-neighbor shifted diffs + 1x1 conv) as a single 3x3 conv,
run data-parallel across 8 TRN2 NeuronCores via Bass/Tile.

Math: out[o,h,w] = sum_k sum_c W[o,3k+c] * (xpad[c,h+ik,w+jk] - x[c,h,w]) + b[o]
    = sum_{c,i,j} K3[o,c,i,j] * xpad[c,h+i,w+j] + b[o]
  where K3 gets W at the 8 non-center taps and -sum(W over taps) at center.

v17 (fp8e3 output + 3-engine drains + p-state pacing; from v16's 146us):
  - OUTPUT in fp8e3 (e3m4). Measured end-to-end rel err 1.6e-2 < 2e-2
    budget (fp16 in / fp32 psum / e3m4 out). Output DMA halves:
    33.5MB -> 16.8MB per core, so DMA stops being the pacer.
  - With DMA slack, the pacer becomes the PSUM drain (fp32->fp8+bias).
    Drains split across THREE engines per chunk: DVE q0 (1.26us),
    ACT q1+q2 (2.1us), Pool q3 (1.5us) -> ~2.3us/chunk wall.
  - PE p-state: matmuls run 427ns/512col in the mid power state but
    213ns after >3us of GAP-FREE execution (hw ramps to 2.4GHz). 8 real
    matmuls/chunk at max rate (1.7us) would outrun the drains and
    stall (resetting to mid). TWO zero-weight K=1 dummy matmuls per
    chunk (start=False into an already-computed bank, +0 numerically)
    pace PE at 10*213=2.13us/chunk, just under the drain pace, so PE
    stays gap-free (ring slack absorbs drift) and holds max p-state.
  - Head: chunk 0's full 82-partition block (j0|zeros|j2|zeros|j1) is
    pre-built by the host -> ONE gpsimd DMA, no j2-copy dependency.
    Weight/bias DMAs issue on vector/scalar in parallel. v16's head was
    14.7us (wsb 256B-packet transfer + serialized gpsimd issues).
  - Steady-state input DMAs (j0,j1,zeros) + output DMA issue on sync
    (gpsimd now drains q3; its queue must stay clear).
  - Partition map unchanged from v16 (K=82): j0@0..17, zeros@18..31 and
    50..63, j2@32..49 (4x DVE copy, 32-aligned), j1@64..81 (odd SDMA
    engines for input/output engine balance).
"""

import numpy as np
import ml_dtypes

import concourse.bacc as bacc
import concourse.bass as bass
import concourse.tile as tile
from concourse import mybir
from concourse.bass_utils import run_bass_kernel_spmd

# Problem constants (hardcoded per harness contract)
B, C, H, W_DIM, OUT = 16, 3, 512, 512, 32
KS, P = 3, 1
NCORES = 8
NB = B // NCORES          # images per core = 2
Hp, Wp = H + 2 * P, W_DIM + 2 * P   # 514, 514

G = 4                     # output rows per matmul group
S = G + KS - 1            # input rows per group window = 6
T = 4                     # groups per PSUM half
TB = 8                    # groups per chunk (32 output rows)
CH = G * TB               # 32 output rows per chunk
NCHUNK = H // CH          # 16 chunks per image
K0 = C * S                # 18 contraction partitions per j-block
GAP = 14                  # zero partitions 18..31 and 50..63 (j2 starts at 32)
OFF_J = (0, 64, 32)       # partition start of the j=0,1,2 blocks
K = 82                    # total contraction partitions incl. zero gaps
M = G * OUT               # 128 output partitions
FW = TB * Wp              # 4112 free cols per contraction row
OBF = 2 * T * W_DIM       # 4096 free cols in the output tile
XBUFS = 8                 # xin pool depth (gap partitions zeroed once per buffer)
LA_DMA = 6                # chunks of input-DMA lookahead
LA_J2 = 3                 # chunks of j2-replica lookahead
HB = TB // 2              # groups per PSUM half-tile (4 banks; 2 tiles in flight)
DSPLIT = 1408             # DVE/ACT drain column split within half A
ND_HALF = 2               # zero-weight pacing matmuls per half-chunk

F32 = mybir.dt.float32
F16 = mybir.dt.float16
F8 = mybir.dt.float8e3    # e3m4: output storage dtype


def _coords():
    i, j = np.meshgrid(np.arange(KS), np.arange(KS))
    coords = np.dstack((i.reshape(-1), j.reshape(-1)))[0]
    return coords[np.any(coords != P, axis=1)]


def _build_weights(W, b):
    K3 = np.zeros((OUT, C, KS, KS), np.float32)
    Wr = W.reshape(OUT, 8, C)
    for k, (i, j) in enumerate(_coords()):
        K3[:, :, i, j] += Wr[:, k, :]
    K3[:, :, P, P] = -Wr.sum(axis=1)

    # wt[OFF_J[j] + Sc + s, 32g + o] = K3[o, c, s-g, j] when 0 <= s-g < KS
    wt = np.zeros((K, M), np.float32)
    for j in range(KS):
        for c in range(C):
            for s in range(S):
                for g in range(G):
                    i = s - g
                    if 0 <= i < KS:
                        wt[OFF_J[j] + S * c + s, OUT * g: OUT * (g + 1)] = K3[:, c, i, j]
    bias = np.tile(b.astype(np.float32), G).reshape(M, 1)
    return wt.astype(np.float16), bias


def _build_xin(x):
    """[B,C,H,W] -> j0/j1 im2row blocks [B, NCHUNK, 2, K0, FW] fp16, plus the
    full 82-partition chunk-0 block [B, K, FW] (j0|0|j2|0|j1) for the head."""
    xpad = np.pad(np.ascontiguousarray(x, np.float32),
                  ((0, 0), (0, 0), (P, P), (P, P))).astype(np.float16)
    ch = np.arange(NCHUNK)[:, None, None]
    s = np.arange(S)[None, :, None]
    t = np.arange(TB)[None, None, :]
    rows = CH * ch + G * t + s                      # [NCHUNK, S, TB]
    big = xpad[:, :, rows, :]                       # [B, C, NCHUNK, S, TB, Wp]
    big = big.transpose(0, 2, 1, 3, 4, 5)           # [B, NCHUNK, C, S, TB, Wp]
    j0 = np.ascontiguousarray(big).reshape(B, NCHUNK, K0, FW)
    xin = np.zeros((B, NCHUNK, 2, K0, FW), np.float16)
    xin[:, :, 0] = j0
    xin[:, :, 1, :, :FW - 1] = j0[:, :, :, 1:]
    # chunk-0 full block: one DMA covers j0+gaps+j2+j1 so the first matmul
    # depends on a single transfer (no on-chip j2 copy, no separate zeros).
    xin0 = np.zeros((B, K, FW), np.float16)
    xin0[:, 0:K0] = j0[:, 0]
    xin0[:, 32:32 + K0, :FW - 2] = j0[:, 0, :, 2:]
    xin0[:, 64:64 + K0] = xin[:, 0, 1]
    return xin, xin0


def _build_bass():
    # Bacc (not plain Bass): its compile() runs move_matmul_waits_to_ldweights
    # and generate_event_semaphores, required because TRN2 instructions take
    # at most one semaphore wait.
    nc = bacc.Bacc("TRN2")
    x_d = nc.declare_dram_parameter("xin", [NB, NCHUNK, 2, K0, FW], F16, isOutput=False)
    x0_d = nc.declare_dram_parameter("xin0", [NB, K, FW], F16, isOutput=False)
    wt_d = nc.declare_dram_parameter("wt", [K, M], F16, isOutput=False)
    b_d = nc.declare_dram_parameter("bias", [M, 1], F32, isOutput=False)
    z_d = nc.declare_dram_parameter("zeros", [GAP, FW], F16, isOutput=False)
    out_d = nc.declare_dram_parameter("out", [NB, NCHUNK, M, OBF], F8, isOutput=True)

    with tile.TileContext(nc) as tc:
        with (
            tc.tile_pool(name="wpool", bufs=1) as wpool,
            tc.tile_pool(name="xpool", bufs=XBUFS) as xpool,
            tc.tile_pool(name="opool", bufs=6) as opool,
            tc.tile_pool(name="psum", bufs=2, space=bass.MemorySpace.PSUM) as ppool,
        ):
            # Weights first on the sync queue, bias on scalar: both issue in
            # parallel with the gpsimd chunk-0 input DMA, ahead of everything.
            wsb = wpool.tile([K, M], F16)
            nc.sync.dma_start(wsb[:], wt_d[:])
            bsb = wpool.tile([M, 1], F32)
            nc.scalar.dma_start(bsb[:], b_d[:])
            # Zero weight row for the pacing dummies (must start at
            # partition 0: Ldweights for a (0,0) PE tile requires it).
            zrow = wpool.tile([1, M], F16)
            nc.vector.memset(zrow[:], 0.0)

            zsrc = bass.AP(z_d, 0, [[FW, GAP], [1, FW]])

            def dma_stage(idx):
                """Issue input DMAs for chunk idx; return tile.

                Chunk 0 is ONE transfer of the host-prebuilt 82-partition
                block on gpsimd/SWDGE (head critical path). Later chunks
                issue j0/j1 (+ one-time gap zeros) on sync/HWDGE, keeping
                the gpsimd queue free for its PSUM drain duty.
                """
                xin = xpool.tile([K, FW], F16)
                if idx == 0:
                    nc.gpsimd.dma_start(xin[:], bass.AP(x0_d, 0, [[FW, K], [1, FW]]))
                    return xin
                base = idx * 2 * K0 * FW
                src0 = bass.AP(x_d, base, [[FW, K0], [1, FW]])
                src1 = bass.AP(x_d, base + K0 * FW, [[FW, K0], [1, FW]])
                nc.sync.dma_start(xin[:K0, :], src0)
                if idx < XBUFS:
                    nc.sync.dma_start(xin[K0:K0 + GAP, :], zsrc)
                    nc.sync.dma_start(xin[50:50 + GAP, :], zsrc)
                nc.sync.dma_start(xin[64:64 + K0, :], src1)
                return xin

            def j2_stage(xin):
                # j=2 replica: j0 shifted left 2 cols (4B-aligned -> 4x DVE).
                # Issued LA_J2 chunks ahead of compute but LA_DMA-LA_J2 chunks
                # AFTER its input DMA was issued (in-order engine queue: must
                # not reach the queue head while its DMA is in flight).
                nc.vector.tensor_scalar_add(
                    xin[32: 32 + K0, : FW - 2], xin[:K0, 2:], 0.0,
                )

            TOTAL = NB * NCHUNK
            tiles = [dma_stage(i) for i in range(min(LA_DMA, TOTAL))]
            for i in range(1, min(LA_J2, TOTAL)):
                j2_stage(tiles[i])
            for idx in range(TOTAL):
                if idx + LA_DMA < TOTAL:
                    tiles.append(dma_stage(idx + LA_DMA))
                xin = tiles[idx]

                ob = opool.tile([M, 2, HB * W_DIM], F8)
                for half in range(2):
                    # Flat 4-bank PSUM tile: lets the drain split at a
                    # non-bank-aligned column for DVE/ACT load balance.
                    ps = ppool.tile([M, HB * W_DIM], F32)
                    for i in range(HB):
                        t = half * HB + i
                        # Pacing dummies ride the last mm of each half: it
                        # keeps its accumulation group open (stop=False) and
                        # zero-weight K=1 matmuls add +0 into the same bank,
                        # closing the group. Keeps PE issue rate just under
                        # the drain pace so PE never gaps (max p-state).
                        pace = ND_HALF if i == HB - 1 else 0
                        nc.tensor.matmul(
                            ps[:, W_DIM * i: W_DIM * (i + 1)],
                            wsb[:],
                            xin[:, Wp * t: Wp * t + W_DIM],
                            start=True,
                            stop=not pace,
                        )
                        for d in range(pace):
                            nc.tensor.matmul(
                                ps[:, W_DIM * i: W_DIM * (i + 1)],
                                zrow[:],
                                xin[0:1, Wp * t: Wp * t + W_DIM],
                                start=False,
                                stop=d == pace - 1,
                                tile_position=(0, 0),
                            )
                    # PSUM drain + bias + fp8e3 convert. GPSIMD cannot touch
                    # PSUM (BIR verifier), so split across DVE and ACT with
                    # DVE's share shrunk to make room for its j2 copy:
                    # DVE = A[0:DSPLIT] + j2 (~2.7us), ACT = A[DSPLIT:] +
                    # all of B (~2.7us).
                    if half == 0:
                        nc.vector.tensor_scalar_add(
                            ob[:, 0, :DSPLIT], ps[:, :DSPLIT], bsb[:],
                        )
                        # j2 for a future chunk goes on the DVE queue directly
                        # after this chunk's DVE drain (v16 ordering: keeps
                        # the next-chunk mm WAR from transitively waiting)
                        if idx + LA_J2 < TOTAL:
                            j2_stage(tiles[idx + LA_J2])
                        nc.scalar.add(
                            ob[:, 0, DSPLIT:], ps[:, DSPLIT:], bsb[:],
                        )
                    else:
                        nc.scalar.add(
                            ob[:, 1, :], ps[:, :], bsb[:],
                        )

                dst = bass.AP(
                    out_d,
                    idx * M * OBF,
                    [[OBF, M], [1, OBF]],
                )
                nc.sync.dma_start(dst, ob[:])

    nc.finalize()
    return nc


_NC_CACHE = None


def _get_nc():
    global _NC_CACHE
    if _NC_CACHE is None:
        _NC_CACHE = _build_bass()
    return _NC_CACHE


def kernel(x, W, b, trace=False, **trace_kw):
    xin, xin0 = _build_xin(np.asarray(x, np.float32))
    wt, bias = _build_weights(np.asarray(W, np.float32), np.asarray(b, np.float32))
    zeros = np.zeros((GAP, FW), np.float16)
    in_maps = [
        {"xin": xin[NB * m: NB * (m + 1)], "xin0": xin0[NB * m: NB * (m + 1)],
         "wt": wt, "bias": bias, "zeros": zeros}
        for m in range(NCORES)
    ]
    res = run_bass_kernel_spmd(
        _get_nc(), in_maps, list(range(NCORES)), trace=trace, **trace_kw
    )
    # Device layout [NB, NCHUNK, 32g+o, (half,t4,w)] -> [B, OUT, H, W]:
    # row = CH*chunk + 4*(4*half + t4) + g
    parts = []
    for m in range(NCORES):
        o = res.results[m]["out"]
        o = np.asarray(o).view(ml_dtypes.float8_e3m4).astype(np.float32)
        o = o.reshape(NB, NCHUNK, G, OUT, 2, T, W_DIM)
        parts.append(o.transpose(0, 3, 1, 4, 5, 2, 6).reshape(NB, OUT, H, W_DIM))
    out = np.ascontiguousarray(np.concatenate(parts, axis=0))
    if trace:
        kernel.last_results = res
    return out


# revision 10
# speedup vs baseline: 1.1687x; 1.1687x over previous
"""PointConvolution (8-neighbor shifted diffs + 1x1 conv) as a single 3x3 conv,
run data-parallel across 8 TRN2 NeuronCores via Bass/Tile.

Math: out[o,h,w] = sum_k sum_c W[o,3k+c] * (xpad[c,h+ik,w+jk] - x[c,h,w]) + b[o]
    = sum_{c,i,j} K3[o,c,i,j] * xpad[c,h+i,w+j] + b[o]
  where K3 gets W at the 8 non-center taps and -sum(W over taps) at center.

v17 (fp8e3 output + 3-engine drains + p-state pacing; from v16's 146us):
  - OUTPUT in fp8e3 (e3m4). Measured end-to-end rel err 1.6e-2 < 2e-2
    budget (fp16 in / fp32 psum / e3m4 out). Output DMA halves:
    33.5MB -> 16.8MB per core, so DMA stops being the pacer.
  - With DMA slack, the pacer becomes the PSUM drain (fp32->fp8+bias).
    Drains split across THREE engines per chunk: DVE q0 (1.26us),
    ACT q1+q2 (2.1us), Pool q3 (1.5us) -> ~2.3us/chunk wall.
  - PE p-state: matmuls run 427ns/512col in the mid power state but
    213ns after >3us of GAP-FREE execution (hw ramps to 2.4GHz). 8 real
    matmuls/chunk at max rate (1.7us) would outrun the drains and
    stall (resetting to mid). TWO zero-weight K=1 dummy matmuls per
    chunk (start=False into an already-computed bank, +0 numerically)
    pace PE at 10*213=2.13us/chunk, just under the drain pace, so PE
    stays gap-free (ring slack absorbs drift) and holds max p-state.
  - Head: chunk 0's full 82-partition block (j0|zeros|j2|zeros|j1) is
    pre-built by the host -> ONE gpsimd DMA, no j2-copy dependency.
    Weight/bias DMAs issue on vector/scalar in parallel. v16's head was
    14.7us (wsb 256B-packet transfer + serialized gpsimd issues).
  - Steady-state input DMAs (j0,j1,zeros) + output DMA issue on sync
    (gpsimd now drains q3; its queue must stay clear).
  - Partition map unchanged from v16 (K=82): j0@0..17, zeros@18..31 and
    50..63, j2@32..49 (4x DVE copy, 32-aligned), j1@64..81 (odd SDMA
    engines for input/output engine balance).
"""

import numpy as np
import ml_dtypes

import concourse.bacc as bacc
import concourse.bass as bass
import concourse.tile as tile
from concourse import mybir
from concourse.bass_utils import run_bass_kernel_spmd

# Problem constants (hardcoded per harness contract)
B, C, H, W_DIM, OUT = 16, 3, 512, 512, 32
KS, P = 3, 1
NCORES = 8
NB = B // NCORES          # images per core = 2
Hp, Wp = H + 2 * P, W_DIM + 2 * P   # 514, 514

G = 4                     # output rows per matmul group
S = G + KS - 1            # input rows per group window = 6
T = 4                     # groups per PSUM half
TB = 8                    # groups per chunk (32 output rows)
CH = G * TB               # 32 output rows per chunk
NCHUNK = H // CH          # 16 chunks per image
K0 = C * S                # 18 contraction partitions per j-block
GAP = 14                  # zero partitions 18..31 and 50..63 (j2 starts at 32)
OFF_J = (0, 64, 32)       # partition start of the j=0,1,2 blocks
K = 82                    # total contraction partitions incl. zero gaps
M = G * OUT               # 128 output partitions
FW = TB * Wp              # 4112 free cols per contraction row
OBF = 2 * T * W_DIM       # 4096 free cols in the output tile
XBUFS = 8                 # xin pool depth (gap partitions zeroed once per buffer)
LA_DMA = 6                # chunks of input-DMA lookahead
LA_J2 = 3                 # chunks of j2-replica lookahead
HB = TB // 2              # groups per PSUM half-tile (4 banks; 2 tiles in flight)
DSPLIT = 1408             # DVE/ACT drain column split within half A
ND_HALF = 0               # zero-weight pacing matmuls per half-chunk

F32 = mybir.dt.float32
F16 = mybir.dt.float16
F8 = mybir.dt.float8e3    # e3m4: output storage dtype


def _coords():
    i, j = np.meshgrid(np.arange(KS), np.arange(KS))
    coords = np.dstack((i.reshape(-1), j.reshape(-1)))[0]
    return coords[np.any(coords != P, axis=1)]


def _build_weights(W, b):
    K3 = np.zeros((OUT, C, KS, KS), np.float32)
    Wr = W.reshape(OUT, 8, C)
    for k, (i, j) in enumerate(_coords()):
        K3[:, :, i, j] += Wr[:, k, :]
    K3[:, :, P, P] = -Wr.sum(axis=1)

    # wt[OFF_J[j] + Sc + s, 32g + o] = K3[o, c, s-g, j] when 0 <= s-g < KS
    wt = np.zeros((K, M), np.float32)
    for j in range(KS):
        for c in range(C):
            for s in range(S):
                for g in range(G):
                    i = s - g
                    if 0 <= i < KS:
                        wt[OFF_J[j] + S * c + s, OUT * g: OUT * (g + 1)] = K3[:, c, i, j]
    bias = np.tile(b.astype(np.float32), G).reshape(M, 1)
    return wt.astype(np.float16), bias


def _build_xin(x):
    """[B,C,H,W] -> j0/j1 im2row blocks [B, NCHUNK, 2, K0, FW] fp16, plus the
    full 82-partition chunk-0 block [B, K, FW] (j0|0|j2|0|j1) for the head."""
    xpad = np.pad(np.ascontiguousarray(x, np.float32),
                  ((0, 0), (0, 0), (P, P), (P, P))).astype(np.float16)
    ch = np.arange(NCHUNK)[:, None, None]
    s = np.arange(S)[None, :, None]
    t = np.arange(TB)[None, None, :]
    rows = CH * ch + G * t + s                      # [NCHUNK, S, TB]
    big = xpad[:, :, rows, :]                       # [B, C, NCHUNK, S, TB, Wp]
    big = big.transpose(0, 2, 1, 3, 4, 5)           # [B, NCHUNK, C, S, TB, Wp]
    j0 = np.ascontiguousarray(big).reshape(B, NCHUNK, K0, FW)
    xin = np.zeros((B, NCHUNK, 2, K0, FW), np.float16)
    xin[:, :, 0] = j0
    xin[:, :, 1, :, :FW - 1] = j0[:, :, :, 1:]
    # chunk-0 full block: one DMA covers j0+gaps+j2+j1 so the first matmul
    # depends on a single transfer (no on-chip j2 copy, no separate zeros).
    xin0 = np.zeros((B, K, FW), np.float16)
    xin0[:, 0:K0] = j0[:, 0]
    xin0[:, 32:32 + K0, :FW - 2] = j0[:, 0, :, 2:]
    xin0[:, 64:64 + K0] = xin[:, 0, 1]
    return xin, xin0


def _build_bass():
    # Bacc (not plain Bass): its compile() runs move_matmul_waits_to_ldweights
    # and generate_event_semaphores, required because TRN2 instructions take
    # at most one semaphore wait.
    nc = bacc.Bacc("TRN2")
    x_d = nc.declare_dram_parameter("xin", [NB, NCHUNK, 2, K0, FW], F16, isOutput=False)
    x0_d = nc.declare_dram_parameter("xin0", [NB, K, FW], F16, isOutput=False)
    wt_d = nc.declare_dram_parameter("wt", [K, M], F16, isOutput=False)
    b_d = nc.declare_dram_parameter("bias", [M, 1], F32, isOutput=False)
    z_d = nc.declare_dram_parameter("zeros", [GAP, FW], F16, isOutput=False)
    out_d = nc.declare_dram_parameter("out", [NB, NCHUNK, M, OBF], F8, isOutput=True)

    with tile.TileContext(nc) as tc:
        with (
            tc.tile_pool(name="wpool", bufs=1) as wpool,
            tc.tile_pool(name="xpool", bufs=XBUFS) as xpool,
            tc.tile_pool(name="opool", bufs=6) as opool,
            tc.tile_pool(name="psum", bufs=2, space=bass.MemorySpace.PSUM) as ppool,
        ):
            # Weights first on the sync queue, bias on scalar: both issue in
            # parallel with the gpsimd chunk-0 input DMA, ahead of everything.
            wsb = wpool.tile([K, M], F16)
            nc.sync.dma_start(wsb[:], wt_d[:])
            bsb = wpool.tile([M, 1], F32)
            nc.scalar.dma_start(bsb[:], b_d[:])
            # Zero weight row for the pacing dummies (must start at
            # partition 0: Ldweights for a (0,0) PE tile requires it).
            zrow = wpool.tile([1, M], F16)
            nc.vector.memset(zrow[:], 0.0)

            zsrc = bass.AP(z_d, 0, [[FW, GAP], [1, FW]])

            def dma_stage(idx):
                """Issue input DMAs for chunk idx; return tile.

                Chunk 0 is ONE transfer of the host-prebuilt 82-partition
                block (head critical path). All input issues ride gpsimd /
                SWDGE: they must NOT share the in-order sync queue with the
                drain-gated output issues (that collapses the prefetch).
                """
                xin = xpool.tile([K, FW], F16)
                if idx == 0:
                    nc.gpsimd.dma_start(xin[:], bass.AP(x0_d, 0, [[FW, K], [1, FW]]))
                    return xin
                base = idx * 2 * K0 * FW
                src0 = bass.AP(x_d, base, [[FW, K0], [1, FW]])
                src1 = bass.AP(x_d, base + K0 * FW, [[FW, K0], [1, FW]])
                nc.gpsimd.dma_start(xin[:K0, :], src0)
                if idx < XBUFS:
                    nc.gpsimd.dma_start(xin[K0:K0 + GAP, :], zsrc)
                    nc.gpsimd.dma_start(xin[50:50 + GAP, :], zsrc)
                nc.gpsimd.dma_start(xin[64:64 + K0, :], src1)
                return xin

            def j2_stage(xin):
                # j=2 replica: j0 shifted left 2 cols (4B-aligned -> 4x DVE).
                # Issued LA_J2 chunks ahead of compute but LA_DMA-LA_J2 chunks
                # AFTER its input DMA was issued (in-order engine queue: must
                # not reach the queue head while its DMA is in flight).
                nc.vector.tensor_scalar_add(
                    xin[32: 32 + K0, : FW - 2], xin[:K0, 2:], 0.0,
                )

            TOTAL = NB * NCHUNK
            tiles = [dma_stage(i) for i in range(min(LA_DMA, TOTAL))]
            for i in range(1, min(LA_J2, TOTAL)):
                j2_stage(tiles[i])
            for idx in range(TOTAL):
                if idx + LA_DMA < TOTAL:
                    tiles.append(dma_stage(idx + LA_DMA))
                xin = tiles[idx]

                ob = opool.tile([M, 2, HB * W_DIM], F8)
                for half in range(2):
                    # Flat 4-bank PSUM tile: lets the drain split at a
                    # non-bank-aligned column for DVE/ACT load balance.
                    ps = ppool.tile([M, HB * W_DIM], F32)
                    for i in range(HB):
                        t = half * HB + i
                        # Pacing dummies ride the last mm of each half: it
                        # keeps its accumulation group open (stop=False) and
                        # zero-weight K=1 matmuls add +0 into the same bank,
                        # closing the group. Keeps PE issue rate just under
                        # the drain pace so PE never gaps (max p-state).
                        pace = ND_HALF if i == HB - 1 else 0
                        nc.tensor.matmul(
                            ps[:, W_DIM * i: W_DIM * (i + 1)],
                            wsb[:],
                            xin[:, Wp * t: Wp * t + W_DIM],
                            start=True,
                            stop=not pace,
                        )
                        for d in range(pace):
                            nc.tensor.matmul(
                                ps[:, W_DIM * i: W_DIM * (i + 1)],
                                zrow[:],
                                xin[0:1, Wp * t: Wp * t + W_DIM],
                                start=False,
                                stop=d == pace - 1,
                                tile_position=(0, 0),
                            )
                    # PSUM drain + bias + fp8e3 convert. GPSIMD cannot touch
                    # PSUM (BIR verifier), so split across DVE and ACT with
                    # DVE's share shrunk to make room for its j2 copy:
                    # DVE = A[0:DSPLIT] + j2 (~2.7us), ACT = A[DSPLIT:] +
                    # all of B (~2.7us).
                    if half == 0:
                        nc.vector.tensor_scalar_add(
                            ob[:, 0, :DSPLIT], ps[:, :DSPLIT], bsb[:],
                        )
                        # j2 for a future chunk goes on the DVE queue directly
                        # after this chunk's DVE drain (v16 ordering: keeps
                        # the next-chunk mm WAR from transitively waiting)
                        if idx + LA_J2 < TOTAL:
                            j2_stage(tiles[idx + LA_J2])
                        nc.scalar.add(
                            ob[:, 0, DSPLIT:], ps[:, DSPLIT:], bsb[:],
                        )
                    else:
                        nc.scalar.add(
                            ob[:, 1, :], ps[:, :], bsb[:],
                        )

                dst = bass.AP(
                    out_d,
                    idx * M * OBF,
                    [[OBF, M], [1, OBF]],
                )
                nc.sync.dma_start(dst, ob[:])

    nc.finalize()
    return nc


_NC_CACHE = None


def _get_nc():
    global _NC_CACHE
    if _NC_CACHE is None:
        _NC_CACHE = _build_bass()
    return _NC_CACHE


def kernel(x, W, b, trace=False, **trace_kw):
    xin, xin0 = _build_xin(np.asarray(x, np.float32))
    wt, bias = _build_weights(np.asarray(W, np.float32), np.asarray(b, np.float32))
    zeros = np.zeros((GAP, FW), np.float16)
    in_maps = [
        {"xin": xin[NB * m: NB * (m + 1)], "xin0": xin0[NB * m: NB * (m + 1)],
         "wt": wt, "bias": bias, "zeros": zeros}
        for m in range(NCORES)
    ]
    res = run_bass_kernel_spmd(
        _get_nc(), in_maps, list(range(NCORES)), trace=trace, **trace_kw
    )
    # Device layout [NB, NCHUNK, 32g+o, (half,t4,w)] -> [B, OUT, H, W]:
    # row = CH*chunk + 4*(4*half + t4) + g
    parts = []
    for m in range(NCORES):
        o = res.results[m]["out"]
        o = np.asarray(o).view(ml_dtypes.float8_e3m4).astype(np.float32)
        o = o.reshape(NB, NCHUNK, G, OUT, 2, T, W_DIM)
        parts.append(o.transpose(0, 3, 1, 4, 5, 2, 6).reshape(NB, OUT, H, W_DIM))
    out = np.ascontiguousarray(np.concatenate(parts, axis=0))
    if trace:
        kernel.last_results = res
    return out


# revision 13
# speedup vs baseline: 1.9180x; 1.6412x over previous
"""PointConvolution (8-neighbor shifted diffs + 1x1 conv) as a single 3x3 conv,
run data-parallel across 8 TRN2 NeuronCores via Bass/Tile.

Math: out[o,h,w] = sum_k sum_c W[o,3k+c] * (xpad[c,h+ik,w+jk] - x[c,h,w]) + b[o]
    = sum_{c,i,j} K3[o,c,i,j] * xpad[c,h+i,w+j] + b[o]
  where K3 gets W at the 8 non-center taps and -sum(W over taps) at center.

v17 (fp8e3 output + 3-engine drains + p-state pacing; from v16's 146us):
  - OUTPUT in fp8e3 (e3m4). Measured end-to-end rel err 1.6e-2 < 2e-2
    budget (fp16 in / fp32 psum / e3m4 out). Output DMA halves:
    33.5MB -> 16.8MB per core, so DMA stops being the pacer.
  - With DMA slack, the pacer becomes the PSUM drain (fp32->fp8+bias).
    Drains split across THREE engines per chunk: DVE q0 (1.26us),
    ACT q1+q2 (2.1us), Pool q3 (1.5us) -> ~2.3us/chunk wall.
  - PE p-state: matmuls run 427ns/512col in the mid power state but
    213ns after >3us of GAP-FREE execution (hw ramps to 2.4GHz). 8 real
    matmuls/chunk at max rate (1.7us) would outrun the drains and
    stall (resetting to mid). TWO zero-weight K=1 dummy matmuls per
    chunk (start=False into an already-computed bank, +0 numerically)
    pace PE at 10*213=2.13us/chunk, just under the drain pace, so PE
    stays gap-free (ring slack absorbs drift) and holds max p-state.
  - Head: chunk 0's full 82-partition block (j0|zeros|j2|zeros|j1) is
    pre-built by the host -> ONE gpsimd DMA, no j2-copy dependency.
    Weight/bias DMAs issue on vector/scalar in parallel. v16's head was
    14.7us (wsb 256B-packet transfer + serialized gpsimd issues).
  - Steady-state input DMAs (j0,j1,zeros) + output DMA issue on sync
    (gpsimd now drains q3; its queue must stay clear).
  - Partition map unchanged from v16 (K=82): j0@0..17, zeros@18..31 and
    50..63, j2@32..49 (4x DVE copy, 32-aligned), j1@64..81 (odd SDMA
    engines for input/output engine balance).
"""

import numpy as np
import ml_dtypes

import concourse.bacc as bacc
import concourse.bass as bass
import concourse.tile as tile
from concourse import mybir
from concourse.bass_utils import run_bass_kernel_spmd

# Problem constants (hardcoded per harness contract)
B, C, H, W_DIM, OUT = 16, 3, 512, 512, 32
KS, P = 3, 1
NCORES = 8
NB = B // NCORES          # images per core = 2
Hp, Wp = H + 2 * P, W_DIM + 2 * P   # 514, 514

G = 4                     # output rows per matmul group
S = G + KS - 1            # input rows per group window = 6
T = 4                     # groups per PSUM half
TB = 8                    # groups per chunk (32 output rows)
CH = G * TB               # 32 output rows per chunk
NCHUNK = H // CH          # 16 chunks per image
K0 = C * S                # 18 contraction partitions per j-block
GAP = 14                  # zero partitions 18..31 and 50..63 (j2 starts at 32)
OFF_J = (0, 64, 32)       # partition start of the j=0,1,2 blocks
K = 82                    # total contraction partitions incl. zero gaps
M = G * OUT               # 128 output partitions
FW = TB * Wp              # 4112 free cols per contraction row
OBF = 2 * T * W_DIM       # 4096 free cols in the output tile
XBUFS = 8                 # xin pool depth (gap partitions zeroed once per buffer)
LA_DMA = 6                # chunks of input-DMA lookahead
LA_J2 = 3                 # chunks of j2-replica lookahead
T2 = 2                    # groups per PSUM tile (2 banks; 4 tiles in flight)
NQ = TB // T2             # PSUM tiles per chunk
N_DUMMY = 0               # zero-weight pacing matmuls per chunk

F32 = mybir.dt.float32
F16 = mybir.dt.float16
F8 = mybir.dt.float8e3    # e3m4: output storage dtype


def _coords():
    i, j = np.meshgrid(np.arange(KS), np.arange(KS))
    coords = np.dstack((i.reshape(-1), j.reshape(-1)))[0]
    return coords[np.any(coords != P, axis=1)]


def _build_weights(W, b):
    K3 = np.zeros((OUT, C, KS, KS), np.float32)
    Wr = W.reshape(OUT, 8, C)
    for k, (i, j) in enumerate(_coords()):
        K3[:, :, i, j] += Wr[:, k, :]
    K3[:, :, P, P] = -Wr.sum(axis=1)

    # wt[OFF_J[j] + Sc + s, 32g + o] = K3[o, c, s-g, j] when 0 <= s-g < KS
    wt = np.zeros((K, M), np.float32)
    for j in range(KS):
        for c in range(C):
            for s in range(S):
                for g in range(G):
                    i = s - g
                    if 0 <= i < KS:
                        wt[OFF_J[j] + S * c + s, OUT * g: OUT * (g + 1)] = K3[:, c, i, j]
    bias = np.tile(b.astype(np.float32), G).reshape(M, 1)
    return wt.astype(np.float16), bias


def _build_xin(x):
    """[B,C,H,W] -> j0/j1 im2row blocks [B, NCHUNK, 2, K0, FW] fp16, plus the
    full 82-partition chunk-0 block [B, K, FW] (j0|0|j2|0|j1) for the head."""
    xpad = np.pad(np.ascontiguousarray(x, np.float32),
                  ((0, 0), (0, 0), (P, P), (P, P))).astype(np.float16)
    ch = np.arange(NCHUNK)[:, None, None]
    s = np.arange(S)[None, :, None]
    t = np.arange(TB)[None, None, :]
    rows = CH * ch + G * t + s                      # [NCHUNK, S, TB]
    big = xpad[:, :, rows, :]                       # [B, C, NCHUNK, S, TB, Wp]
    big = big.transpose(0, 2, 1, 3, 4, 5)           # [B, NCHUNK, C, S, TB, Wp]
    j0 = np.ascontiguousarray(big).reshape(B, NCHUNK, K0, FW)
    xin = np.zeros((B, NCHUNK, 2, K0, FW), np.float16)
    xin[:, :, 0] = j0
    xin[:, :, 1, :, :FW - 1] = j0[:, :, :, 1:]
    # chunk-0 full block: one DMA covers j0+gaps+j2+j1 so the first matmul
    # depends on a single transfer (no on-chip j2 copy, no separate zeros).
    xin0 = np.zeros((B, K, FW), np.float16)
    xin0[:, 0:K0] = j0[:, 0]
    xin0[:, 32:32 + K0, :FW - 2] = j0[:, 0, :, 2:]
    xin0[:, 64:64 + K0] = xin[:, 0, 1]
    return xin, xin0


def _build_bass():
    # Bacc (not plain Bass): its compile() runs move_matmul_waits_to_ldweights
    # and generate_event_semaphores, required because TRN2 instructions take
    # at most one semaphore wait.
    nc = bacc.Bacc("TRN2")
    x_d = nc.declare_dram_parameter("xin", [NB, NCHUNK, 2, K0, FW], F16, isOutput=False)
    x0_d = nc.declare_dram_parameter("xin0", [NB, K, FW], F16, isOutput=False)
    wt_d = nc.declare_dram_parameter("wt", [K, M], F16, isOutput=False)
    b_d = nc.declare_dram_parameter("bias", [M, 1], F32, isOutput=False)
    z_d = nc.declare_dram_parameter("zeros", [GAP, FW], F16, isOutput=False)
    out_d = nc.declare_dram_parameter("out", [NB, NCHUNK, M, OBF], F8, isOutput=True)

    with tile.TileContext(nc) as tc:
        with (
            tc.tile_pool(name="wpool", bufs=1) as wpool,
            tc.tile_pool(name="xpool", bufs=XBUFS) as xpool,
            tc.tile_pool(name="opool", bufs=6) as opool,
            tc.tile_pool(name="psum", bufs=4, space=bass.MemorySpace.PSUM) as ppool,
        ):
            # Weights first on the sync queue, bias on scalar: both issue in
            # parallel with the gpsimd chunk-0 input DMA, ahead of everything.
            wsb = wpool.tile([K, M], F16)
            nc.sync.dma_start(wsb[:], wt_d[:])
            bsb = wpool.tile([M, 1], F32)
            nc.scalar.dma_start(bsb[:], b_d[:])
            # Zero weight row for the pacing dummies (must start at
            # partition 0: Ldweights for a (0,0) PE tile requires it).
            zrow = wpool.tile([1, M], F16)
            nc.vector.memset(zrow[:], 0.0)

            zsrc = bass.AP(z_d, 0, [[FW, GAP], [1, FW]])

            def dma_stage(idx):
                """Issue input DMAs for chunk idx; return tile.

                Chunk 0 is ONE transfer of the host-prebuilt 82-partition
                block (head critical path). All input issues ride gpsimd /
                SWDGE: they must NOT share the in-order sync queue with the
                drain-gated output issues (that collapses the prefetch).
                """
                xin = xpool.tile([K, FW], F16)
                if idx == 0:
                    nc.gpsimd.dma_start(xin[:], bass.AP(x0_d, 0, [[FW, K], [1, FW]]))
                    return xin
                base = idx * 2 * K0 * FW
                src0 = bass.AP(x_d, base, [[FW, K0], [1, FW]])
                src1 = bass.AP(x_d, base + K0 * FW, [[FW, K0], [1, FW]])
                nc.gpsimd.dma_start(xin[:K0, :], src0)
                if idx < XBUFS:
                    nc.gpsimd.dma_start(xin[K0:K0 + GAP, :], zsrc)
                    nc.gpsimd.dma_start(xin[50:50 + GAP, :], zsrc)
                nc.gpsimd.dma_start(xin[64:64 + K0, :], src1)
                return xin

            def j2_stage(xin):
                # j=2 replica: j0 shifted left 2 cols (4B-aligned -> 4x DVE).
                # Issued LA_J2 chunks ahead of compute but LA_DMA-LA_J2 chunks
                # AFTER its input DMA was issued (in-order engine queue: must
                # not reach the queue head while its DMA is in flight).
                nc.vector.tensor_scalar_add(
                    xin[32: 32 + K0, : FW - 2], xin[:K0, 2:], 0.0,
                )

            TOTAL = NB * NCHUNK
            tiles = [dma_stage(i) for i in range(min(LA_DMA, TOTAL))]
            for i in range(1, min(LA_J2, TOTAL)):
                j2_stage(tiles[i])
            for idx in range(TOTAL):
                if idx + LA_DMA < TOTAL:
                    tiles.append(dma_stage(idx + LA_DMA))
                xin = tiles[idx]

                ob = opool.tile([M, NQ, T2, W_DIM], F8)
                for quarter in range(NQ):
                    ps = ppool.tile([M, T2, W_DIM], F32)
                    for t2 in range(T2):
                        t = quarter * T2 + t2
                        # Pacing dummies ride the last mm of the chunk: it
                        # keeps its accumulation group open (stop=False) and
                        # zero-weight K=1 matmuls add +0 into the same bank,
                        # closing the group. Keeps PE issue rate just under
                        # the drain pace so PE never gaps (max p-state).
                        pace = N_DUMMY if quarter == NQ - 1 and t2 == T2 - 1 else 0
                        nc.tensor.matmul(
                            ps[:, t2, :],
                            wsb[:],
                            xin[:, Wp * t: Wp * t + W_DIM],
                            start=True,
                            stop=not pace,
                        )
                        for d in range(pace):
                            nc.tensor.matmul(
                                ps[:, t2, :],
                                zrow[:],
                                xin[0:1, Wp * t: Wp * t + W_DIM],
                                start=False,
                                stop=d == pace - 1,
                                tile_position=(0, 0),
                            )
                    # PSUM drain + bias + fp8e3 convert. GPSIMD cannot touch
                    # PSUM (BIR verifier), and splitting one tile across two
                    # engines makes bacc hoist coarse chunk-start events onto
                    # the PE queue (v17b ran 253us that way). So exactly ONE
                    # engine per 2-bank tile, v16-style: q0 -> DVE (+j2),
                    # q1-q3 -> ACT.
                    if quarter == 0:
                        nc.vector.tensor_scalar_add(
                            ob[:, quarter], ps[:, :, :], bsb[:],
                        )
                        # j2 for a future chunk goes on the DVE queue directly
                        # after this chunk's DVE drain (v16 ordering: keeps
                        # the next-chunk mm WAR from transitively waiting)
                        if idx + LA_J2 < TOTAL:
                            j2_stage(tiles[idx + LA_J2])
                    else:
                        nc.scalar.add(
                            ob[:, quarter], ps[:, :, :], bsb[:],
                        )

                dst = bass.AP(
                    out_d,
                    idx * M * OBF,
                    [[OBF, M], [1, OBF]],
                )
                nc.sync.dma_start(dst, ob[:])

    nc.finalize()
    return nc


_NC_CACHE = None


def _get_nc():
    global _NC_CACHE
    if _NC_CACHE is None:
        _NC_CACHE = _build_bass()
    return _NC_CACHE


def kernel(x, W, b, trace=False, **trace_kw):
    xin, xin0 = _build_xin(np.asarray(x, np.float32))
    wt, bias = _build_weights(np.asarray(W, np.float32), np.asarray(b, np.float32))
    zeros = np.zeros((GAP, FW), np.float16)
    in_maps = [
        {"xin": xin[NB * m: NB * (m + 1)], "xin0": xin0[NB * m: NB * (m + 1)],
         "wt": wt, "bias": bias, "zeros": zeros}
        for m in range(NCORES)
    ]
    res = run_bass_kernel_spmd(
        _get_nc(), in_maps, list(range(NCORES)), trace=trace, **trace_kw
    )
    # Device layout [NB, NCHUNK, 32g+o, (half,t4,w)] -> [B, OUT, H, W]:
    # row = CH*chunk + 4*(4*half + t4) + g
    parts = []
    for m in range(NCORES):
        o = res.results[m]["out"]
        o = np.asarray(o).view(ml_dtypes.float8_e3m4).astype(np.float32)
        o = o.reshape(NB, NCHUNK, G, OUT, 2, T, W_DIM)
        parts.append(o.transpose(0, 3, 1, 4, 5, 2, 6).reshape(NB, OUT, H, W_DIM))
    out = np.ascontiguousarray(np.concatenate(parts, axis=0))
    if trace:
        kernel.last_results = res
    return out


# revision 14
# speedup vs baseline: 2.0305x; 1.0587x over previous
"""PointConvolution (8-neighbor shifted diffs + 1x1 conv) as a single 3x3 conv,
run data-parallel across 8 TRN2 NeuronCores via Bass/Tile.

Math: out[o,h,w] = sum_k sum_c W[o,3k+c] * (xpad[c,h+ik,w+jk] - x[c,h,w]) + b[o]
    = sum_{c,i,j} K3[o,c,i,j] * xpad[c,h+i,w+j] + b[o]
  where K3 gets W at the 8 non-center taps and -sum(W over taps) at center.

v18 (fp8e3 output + all-DRAM im2row + balanced 2-engine drains; from
v16's 146us):
  - OUTPUT in fp8e3 (e3m4): end-to-end rel err 1.6e-2 < 2e-2 budget
    (fp16 in / fp32 psum / e3m4 out). Output DMA halves: 33.5->16.8MB.
  - ALL THREE j-shift im2row blocks ship from DRAM (v16 built j2 with a
    1.07us/chunk DVE copy). With fp8 output the DMA has slack
    (956KB/chunk = 2.6us < drain pace), and dropping the copy frees DVE
    for drains. No copy alignment constraints -> K=54, no gap
    partitions, no zeros DMA.
  - PSUM drains (fp32->fp8e3 + bias): one engine per 2-bank tile
    (splitting a tile across engines makes bacc hoist coarse WAR events
    onto the PE queue -- that cost 100us in v17b): DVE q0,q2 (2.51us),
    ACT q1,q3 (2.22us) per chunk.
  - Pacer: PE at mid p-state (427ns/512-col mm) = 3.41us/chunk, or at
    max p-state (213ns, needs >3us gap-free PE) = 1.71us/chunk with
    N_DUMMY zero-weight pacing matmuls absorbing the difference from
    the drain pace (~2.6). N_DUMMY=0 -> safe mid-state ~3.5us/chunk.
  - Head: a single DMA transfer concentrates on ~2 DMA engines
    (measured: 674KB one-transfer chunk-0 block took 15us in v17d!) so
    chunk 0's three blocks are split into 9 sub-transfers issued from
    the three DMA-capable queues (gpsimd/scalar/sync) in parallel.
  - Steady input issues on gpsimd (dedicated); output DMA on sync;
    weights first on sync; bias on scalar.
"""

import numpy as np
import ml_dtypes

import concourse.bacc as bacc
import concourse.bass as bass
import concourse.tile as tile
from concourse import mybir
from concourse.bass_utils import run_bass_kernel_spmd

# Problem constants (hardcoded per harness contract)
B, C, H, W_DIM, OUT = 16, 3, 512, 512, 32
KS, P = 3, 1
NCORES = 8
NB = B // NCORES          # images per core = 2
Hp, Wp = H + 2 * P, W_DIM + 2 * P   # 514, 514

G = 4                     # output rows per matmul group
S = G + KS - 1            # input rows per group window = 6
T = 4                     # groups per PSUM half
TB = 8                    # groups per chunk (32 output rows)
CH = G * TB               # 32 output rows per chunk
NCHUNK = H // CH          # 16 chunks per image
K0 = C * S                # 18 contraction partitions per j-block
K = 3 * K0                # 54 contraction partitions, no gaps
M = G * OUT               # 128 output partitions
FW = TB * Wp              # 4112 free cols per contraction row
OBF = 2 * T * W_DIM       # 4096 free cols in the output tile
XBUFS = 8                 # xin pool depth
LA_DMA = 6                # chunks of input-DMA lookahead
T2 = 2                    # groups per PSUM tile (2 banks; 4 tiles in flight)
NQ = TB // T2             # PSUM tiles per chunk
N_DUMMY = 0               # zero-weight pacing matmuls per chunk

F32 = mybir.dt.float32
F16 = mybir.dt.float16
F8 = mybir.dt.float8e3    # e3m4: output storage dtype


def _coords():
    i, j = np.meshgrid(np.arange(KS), np.arange(KS))
    coords = np.dstack((i.reshape(-1), j.reshape(-1)))[0]
    return coords[np.any(coords != P, axis=1)]


def _build_weights(W, b):
    K3 = np.zeros((OUT, C, KS, KS), np.float32)
    Wr = W.reshape(OUT, 8, C)
    for k, (i, j) in enumerate(_coords()):
        K3[:, :, i, j] += Wr[:, k, :]
    K3[:, :, P, P] = -Wr.sum(axis=1)

    # wt[K0*j + S*c + s, 32g + o] = K3[o, c, s-g, j] when 0 <= s-g < KS
    wt = np.zeros((K, M), np.float32)
    for j in range(KS):
        for c in range(C):
            for s in range(S):
                for g in range(G):
                    i = s - g
                    if 0 <= i < KS:
                        wt[K0 * j + S * c + s, OUT * g: OUT * (g + 1)] = K3[:, c, i, j]
    bias = np.tile(b.astype(np.float32), G).reshape(M, 1)
    return wt.astype(np.float16), bias


def _build_xin(x):
    """[B,C,H,W] -> [B, NCHUNK, 3, K0, FW] fp16: j=0,1,2 im2row blocks."""
    xpad = np.pad(np.ascontiguousarray(x, np.float32),
                  ((0, 0), (0, 0), (P, P), (P, P))).astype(np.float16)
    ch = np.arange(NCHUNK)[:, None, None]
    s = np.arange(S)[None, :, None]
    t = np.arange(TB)[None, None, :]
    rows = CH * ch + G * t + s                      # [NCHUNK, S, TB]
    big = xpad[:, :, rows, :]                       # [B, C, NCHUNK, S, TB, Wp]
    big = big.transpose(0, 2, 1, 3, 4, 5)           # [B, NCHUNK, C, S, TB, Wp]
    j0 = np.ascontiguousarray(big).reshape(B, NCHUNK, K0, FW)
    xin = np.zeros((B, NCHUNK, 3, K0, FW), np.float16)
    xin[:, :, 0] = j0
    xin[:, :, 1, :, :FW - 1] = j0[:, :, :, 1:]
    xin[:, :, 2, :, :FW - 2] = j0[:, :, :, 2:]
    return xin


def _build_bass():
    # Bacc (not plain Bass): its compile() runs move_matmul_waits_to_ldweights
    # and generate_event_semaphores, required because TRN2 instructions take
    # at most one semaphore wait.
    nc = bacc.Bacc("TRN2")
    x_d = nc.declare_dram_parameter("xin", [NB, NCHUNK, 3, K0, FW], F16, isOutput=False)
    wt_d = nc.declare_dram_parameter("wt", [K, M], F16, isOutput=False)
    b_d = nc.declare_dram_parameter("bias", [M, 1], F32, isOutput=False)
    out_d = nc.declare_dram_parameter("out", [NB, NCHUNK, M, OBF], F8, isOutput=True)

    with tile.TileContext(nc) as tc:
        with (
            tc.tile_pool(name="wpool", bufs=1) as wpool,
            tc.tile_pool(name="xpool", bufs=XBUFS) as xpool,
            tc.tile_pool(name="opool", bufs=6) as opool,
            tc.tile_pool(name="psum", bufs=4, space=bass.MemorySpace.PSUM) as ppool,
        ):
            # Weights first on the sync queue, bias on scalar: both issue in
            # parallel with the gpsimd chunk-0 input DMAs, ahead of the rest.
            wsb = wpool.tile([K, M], F16)
            nc.sync.dma_start(wsb[:], wt_d[:])
            bsb = wpool.tile([M, 1], F32)
            nc.scalar.dma_start(bsb[:], b_d[:])
            # Zero weight row for the pacing dummies (must start at
            # partition 0: Ldweights for a (0,0) PE tile requires it).
            zrow = wpool.tile([1, M], F16)
            nc.vector.memset(zrow[:], 0.0)

            def dma_stage(idx):
                """Issue input DMAs for chunk idx; return tile.

                One transfer concentrates on ~2 DMA engines, so chunk 0 (the
                head critical path) is split into 9 sub-transfers issued from
                all three DMA-capable queues in parallel. Steady chunks are 3
                transfers (one per j block) on gpsimd, whose queue has no
                other duty.
                """
                xin = xpool.tile([K, FW], F16)
                base = idx * 3 * K0 * FW
                if idx == 0:
                    sub = K0 // 3
                    engines = (nc.gpsimd, nc.scalar, nc.sync)
                    for jb in range(3):
                        for piece in range(3):
                            p0 = jb * K0 + piece * sub
                            src = bass.AP(x_d, base + p0 * FW, [[FW, sub], [1, FW]])
                            engines[piece].dma_start(xin[p0:p0 + sub, :], src)
                    return xin
                for jb in range(3):
                    src = bass.AP(x_d, base + jb * K0 * FW, [[FW, K0], [1, FW]])
                    nc.gpsimd.dma_start(xin[jb * K0:(jb + 1) * K0, :], src)
                return xin

            TOTAL = NB * NCHUNK
            tiles = [dma_stage(i) for i in range(min(LA_DMA, TOTAL))]
            for idx in range(TOTAL):
                if idx + LA_DMA < TOTAL:
                    tiles.append(dma_stage(idx + LA_DMA))
                xin = tiles[idx]

                ob = opool.tile([M, NQ, T2, W_DIM], F8)
                for quarter in range(NQ):
                    ps = ppool.tile([M, T2, W_DIM], F32)
                    for t2 in range(T2):
                        t = quarter * T2 + t2
                        # Pacing dummies ride the last mm of each quarter:
                        # it keeps its accumulation group open (stop=False)
                        # and zero-weight K=1 matmuls add +0 into the same
                        # bank, closing the group. Keeps the PE issue rate
                        # just under the drain pace so PE never gaps and
                        # holds max p-state.
                        pace = 0
                        if t2 == T2 - 1:
                            pace = N_DUMMY // NQ
                            if quarter == NQ - 1:
                                pace += N_DUMMY % NQ
                        nc.tensor.matmul(
                            ps[:, t2, :],
                            wsb[:],
                            xin[:, Wp * t: Wp * t + W_DIM],
                            start=True,
                            stop=not pace,
                        )
                        for d in range(pace):
                            nc.tensor.matmul(
                                ps[:, t2, :],
                                zrow[:],
                                xin[0:1, Wp * t: Wp * t + W_DIM],
                                start=False,
                                stop=d == pace - 1,
                                tile_position=(0, 0),
                            )
                    # PSUM drain + bias + fp8e3 convert. Exactly ONE engine
                    # per 2-bank tile (two engines on one tile makes bacc
                    # hoist coarse chunk-start events onto the PE queue):
                    # DVE q0,q2 (2x1.26us), ACT q1,q3 (2x1.11us).
                    if quarter % 2 == 0:
                        nc.vector.tensor_scalar_add(
                            ob[:, quarter], ps[:, :, :], bsb[:],
                        )
                    else:
                        nc.scalar.add(
                            ob[:, quarter], ps[:, :, :], bsb[:],
                        )

                dst = bass.AP(
                    out_d,
                    idx * M * OBF,
                    [[OBF, M], [1, OBF]],
                )
                nc.sync.dma_start(dst, ob[:])

    nc.finalize()
    return nc


_NC_CACHE = None


def _get_nc():
    global _NC_CACHE
    if _NC_CACHE is None:
        _NC_CACHE = _build_bass()
    return _NC_CACHE


def kernel(x, W, b, trace=False, **trace_kw):
    xin = _build_xin(np.asarray(x, np.float32))
    wt, bias = _build_weights(np.asarray(W, np.float32), np.asarray(b, np.float32))
    in_maps = [
        {"xin": xin[NB * m: NB * (m + 1)], "wt": wt, "bias": bias}
        for m in range(NCORES)
    ]
    res = run_bass_kernel_spmd(
        _get_nc(), in_maps, list(range(NCORES)), trace=trace, **trace_kw
    )
    # Device layout [NB, NCHUNK, 32g+o, (half,t4,w)] -> [B, OUT, H, W]:
    # row = CH*chunk + 4*(4*half + t4) + g
    parts = []
    for m in range(NCORES):
        o = res.results[m]["out"]
        o = np.asarray(o).view(ml_dtypes.float8_e3m4).astype(np.float32)
        o = o.reshape(NB, NCHUNK, G, OUT, 2, T, W_DIM)
        parts.append(o.transpose(0, 3, 1, 4, 5, 2, 6).reshape(NB, OUT, H, W_DIM))
    out = np.ascontiguousarray(np.concatenate(parts, axis=0))
    if trace:
        kernel.last_results = res
    return out


# revision 15
# speedup vs baseline: 2.2258x; 1.0962x over previous
"""PointConvolution (8-neighbor shifted diffs + 1x1 conv) as a single 3x3 conv,
run data-parallel across 8 TRN2 NeuronCores via Bass/Tile.

Math: out[o,h,w] = sum_k sum_c W[o,3k+c] * (xpad[c,h+ik,w+jk] - x[c,h,w]) + b[o]
    = sum_{c,i,j} K3[o,c,i,j] * xpad[c,h+i,w+j] + b[o]
  where K3 gets W at the 8 non-center taps and -sum(W over taps) at center.

v18 (fp8e3 output + all-DRAM im2row + balanced 2-engine drains; from
v16's 146us):
  - OUTPUT in fp8e3 (e3m4): end-to-end rel err 1.6e-2 < 2e-2 budget
    (fp16 in / fp32 psum / e3m4 out). Output DMA halves: 33.5->16.8MB.
  - ALL THREE j-shift im2row blocks ship from DRAM (v16 built j2 with a
    1.07us/chunk DVE copy). With fp8 output the DMA has slack
    (956KB/chunk = 2.6us < drain pace), and dropping the copy frees DVE
    for drains. No copy alignment constraints -> K=54, no gap
    partitions, no zeros DMA.
  - PSUM drains (fp32->fp8e3 + bias): one engine per 2-bank tile
    (splitting a tile across engines makes bacc hoist coarse WAR events
    onto the PE queue -- that cost 100us in v17b): DVE q0,q2 (2.51us),
    ACT q1,q3 (2.22us) per chunk.
  - Pacer: PE at mid p-state (427ns/512-col mm) = 3.41us/chunk, or at
    max p-state (213ns, needs >3us gap-free PE) = 1.71us/chunk with
    N_DUMMY zero-weight pacing matmuls absorbing the difference from
    the drain pace (~2.6). N_DUMMY=0 -> safe mid-state ~3.5us/chunk.
  - Head: a single DMA transfer concentrates on ~2 DMA engines
    (measured: 674KB one-transfer chunk-0 block took 15us in v17d!) so
    chunk 0's three blocks are split into 9 sub-transfers issued from
    the three DMA-capable queues (gpsimd/scalar/sync) in parallel.
  - Steady input issues on gpsimd (dedicated); output DMA on sync;
    weights first on sync; bias on scalar.
"""

import numpy as np
import ml_dtypes

import concourse.bacc as bacc
import concourse.bass as bass
import concourse.tile as tile
from concourse import mybir
from concourse.bass_utils import run_bass_kernel_spmd

# Problem constants (hardcoded per harness contract)
B, C, H, W_DIM, OUT = 16, 3, 512, 512, 32
KS, P = 3, 1
NCORES = 8
NB = B // NCORES          # images per core = 2
Hp, Wp = H + 2 * P, W_DIM + 2 * P   # 514, 514

G = 4                     # output rows per matmul group
S = G + KS - 1            # input rows per group window = 6
T = 4                     # groups per PSUM half
TB = 8                    # groups per chunk (32 output rows)
CH = G * TB               # 32 output rows per chunk
NCHUNK = H // CH          # 16 chunks per image
K0 = C * S                # 18 contraction partitions per j-block
K = 3 * K0                # 54 contraction partitions, no gaps
M = G * OUT               # 128 output partitions
FW = TB * Wp              # 4112 free cols per contraction row
OBF = 2 * T * W_DIM       # 4096 free cols in the output tile
XBUFS = 8                 # xin pool depth
LA_DMA = 6                # chunks of input-DMA lookahead
T2 = 2                    # groups per PSUM tile (2 banks; 4 tiles in flight)
NQ = TB // T2             # PSUM tiles per chunk
N_DUMMY = 0               # zero-weight pacing matmuls per chunk

F32 = mybir.dt.float32
F16 = mybir.dt.float16
F8 = mybir.dt.float8e3    # e3m4: output storage dtype


def _coords():
    i, j = np.meshgrid(np.arange(KS), np.arange(KS))
    coords = np.dstack((i.reshape(-1), j.reshape(-1)))[0]
    return coords[np.any(coords != P, axis=1)]


def _build_weights(W, b):
    K3 = np.zeros((OUT, C, KS, KS), np.float32)
    Wr = W.reshape(OUT, 8, C)
    for k, (i, j) in enumerate(_coords()):
        K3[:, :, i, j] += Wr[:, k, :]
    K3[:, :, P, P] = -Wr.sum(axis=1)

    # wt[K0*j + S*c + s, 32g + o] = K3[o, c, s-g, j] when 0 <= s-g < KS
    wt = np.zeros((K, M), np.float32)
    for j in range(KS):
        for c in range(C):
            for s in range(S):
                for g in range(G):
                    i = s - g
                    if 0 <= i < KS:
                        wt[K0 * j + S * c + s, OUT * g: OUT * (g + 1)] = K3[:, c, i, j]
    bias = np.tile(b.astype(np.float32), G).reshape(M, 1)
    return wt.astype(np.float16), bias


def _build_xin(x):
    """[B,C,H,W] -> [B, NCHUNK, 3, K0, FW] fp16: j=0,1,2 im2row blocks."""
    xpad = np.pad(np.ascontiguousarray(x, np.float32),
                  ((0, 0), (0, 0), (P, P), (P, P))).astype(np.float16)
    ch = np.arange(NCHUNK)[:, None, None]
    s = np.arange(S)[None, :, None]
    t = np.arange(TB)[None, None, :]
    rows = CH * ch + G * t + s                      # [NCHUNK, S, TB]
    big = xpad[:, :, rows, :]                       # [B, C, NCHUNK, S, TB, Wp]
    big = big.transpose(0, 2, 1, 3, 4, 5)           # [B, NCHUNK, C, S, TB, Wp]
    j0 = np.ascontiguousarray(big).reshape(B, NCHUNK, K0, FW)
    xin = np.zeros((B, NCHUNK, 3, K0, FW), np.float16)
    xin[:, :, 0] = j0
    xin[:, :, 1, :, :FW - 1] = j0[:, :, :, 1:]
    xin[:, :, 2, :, :FW - 2] = j0[:, :, :, 2:]
    return xin


def _build_bass():
    # Bacc (not plain Bass): its compile() runs move_matmul_waits_to_ldweights
    # and generate_event_semaphores, required because TRN2 instructions take
    # at most one semaphore wait.
    nc = bacc.Bacc("TRN2")
    x_d = nc.declare_dram_parameter("xin", [NB, NCHUNK, 3, K0, FW], F16, isOutput=False)
    wt_d = nc.declare_dram_parameter("wt", [K, M], F16, isOutput=False)
    b_d = nc.declare_dram_parameter("bias", [M, 1], F32, isOutput=False)
    out_d = nc.declare_dram_parameter("out", [NB, NCHUNK, M, OBF], F8, isOutput=True)

    with tile.TileContext(nc) as tc:
        with (
            tc.tile_pool(name="wpool", bufs=1) as wpool,
            tc.tile_pool(name="xpool", bufs=XBUFS) as xpool,
            tc.tile_pool(name="opool", bufs=6) as opool,
            tc.tile_pool(name="psum", bufs=4, space=bass.MemorySpace.PSUM) as ppool,
        ):
            # Weights first on the sync queue, bias on scalar: both issue in
            # parallel with the gpsimd chunk-0 input DMAs, ahead of the rest.
            wsb = wpool.tile([K, M], F16)
            nc.sync.dma_start(wsb[:], wt_d[:])
            bsb = wpool.tile([M, 1], F32)
            nc.scalar.dma_start(bsb[:], b_d[:])
            # Zero weight row for the pacing dummies (must start at
            # partition 0: Ldweights for a (0,0) PE tile requires it).
            zrow = wpool.tile([1, M], F16)
            nc.vector.memset(zrow[:], 0.0)

            def dma_stage(idx):
                """Issue input DMAs for chunk idx; return tile.

                One transfer concentrates on ~2 DMA engines, so chunk 0 (the
                head critical path) is split into 9 sub-transfers issued from
                all three DMA-capable queues in parallel. Steady chunks are 3
                transfers (one per j block) on gpsimd, whose queue has no
                other duty.
                """
                xin = xpool.tile([K, FW], F16)
                base = idx * 3 * K0 * FW
                if idx == 0:
                    # 6 sub-transfers of 9 partitions: 4 on gpsimd, 2 on sync
                    # (after wsb). NOT on scalar: its HWDGE queue fragments
                    # into ~826B packets and crawls (measured 9us for 148KB).
                    sub = K // 6
                    engines = (nc.gpsimd, nc.gpsimd, nc.gpsimd,
                               nc.gpsimd, nc.sync, nc.sync)
                    for piece in range(6):
                        p0 = piece * sub
                        src = bass.AP(x_d, base + p0 * FW, [[FW, sub], [1, FW]])
                        engines[piece].dma_start(xin[p0:p0 + sub, :], src)
                    return xin
                for jb in range(3):
                    src = bass.AP(x_d, base + jb * K0 * FW, [[FW, K0], [1, FW]])
                    nc.gpsimd.dma_start(xin[jb * K0:(jb + 1) * K0, :], src)
                return xin

            TOTAL = NB * NCHUNK
            tiles = [dma_stage(i) for i in range(min(LA_DMA, TOTAL))]
            for idx in range(TOTAL):
                if idx + LA_DMA < TOTAL:
                    tiles.append(dma_stage(idx + LA_DMA))
                xin = tiles[idx]

                ob = opool.tile([M, NQ, T2, W_DIM], F8)
                for quarter in range(NQ):
                    ps = ppool.tile([M, T2, W_DIM], F32)
                    for t2 in range(T2):
                        t = quarter * T2 + t2
                        # Pacing dummies ride the last mm of each quarter:
                        # it keeps its accumulation group open (stop=False)
                        # and zero-weight K=1 matmuls add +0 into the same
                        # bank, closing the group. Keeps the PE issue rate
                        # just under the drain pace so PE never gaps and
                        # holds max p-state.
                        pace = 0
                        if t2 == T2 - 1:
                            pace = N_DUMMY // NQ
                            if quarter == NQ - 1:
                                pace += N_DUMMY % NQ
                        nc.tensor.matmul(
                            ps[:, t2, :],
                            wsb[:],
                            xin[:, Wp * t: Wp * t + W_DIM],
                            start=True,
                            stop=not pace,
                        )
                        for d in range(pace):
                            nc.tensor.matmul(
                                ps[:, t2, :],
                                zrow[:],
                                xin[0:1, Wp * t: Wp * t + W_DIM],
                                start=False,
                                stop=d == pace - 1,
                                tile_position=(0, 0),
                            )
                    # PSUM drain + bias + fp8e3 convert. Exactly ONE engine
                    # per 2-bank tile (two engines on one tile makes bacc
                    # hoist coarse chunk-start events onto the PE queue):
                    # DVE q0,q2 (2x1.26us), ACT q1,q3 (2x1.11us).
                    if quarter % 2 == 0:
                        nc.vector.tensor_scalar_add(
                            ob[:, quarter], ps[:, :, :], bsb[:],
                        )
                    else:
                        nc.scalar.add(
                            ob[:, quarter], ps[:, :, :], bsb[:],
                        )

                dst = bass.AP(
                    out_d,
                    idx * M * OBF,
                    [[OBF, M], [1, OBF]],
                )
                nc.sync.dma_start(dst, ob[:])

    nc.finalize()
    return nc


_NC_CACHE = None


def _get_nc():
    global _NC_CACHE
    if _NC_CACHE is None:
        _NC_CACHE = _build_bass()
    return _NC_CACHE


def kernel(x, W, b, trace=False, **trace_kw):
    xin = _build_xin(np.asarray(x, np.float32))
    wt, bias = _build_weights(np.asarray(W, np.float32), np.asarray(b, np.float32))
    in_maps = [
        {"xin": xin[NB * m: NB * (m + 1)], "wt": wt, "bias": bias}
        for m in range(NCORES)
    ]
    res = run_bass_kernel_spmd(
        _get_nc(), in_maps, list(range(NCORES)), trace=trace, **trace_kw
    )
    # Device layout [NB, NCHUNK, 32g+o, (half,t4,w)] -> [B, OUT, H, W]:
    # row = CH*chunk + 4*(4*half + t4) + g
    parts = []
    for m in range(NCORES):
        o = res.results[m]["out"]
        o = np.asarray(o).view(ml_dtypes.float8_e3m4).astype(np.float32)
        o = o.reshape(NB, NCHUNK, G, OUT, 2, T, W_DIM)
        parts.append(o.transpose(0, 3, 1, 4, 5, 2, 6).reshape(NB, OUT, H, W_DIM))
    out = np.ascontiguousarray(np.concatenate(parts, axis=0))
    if trace:
        kernel.last_results = res
    return out
